# revision 37
# baseline (speedup 1.0000x reference)
"""Trainium2 Bass kernel for masked cross-attention (sparse_attention).

Reference computation (per batch b):
    q = x @ Wq + bq                      # [N, hd]   (hd = 8 heads * 32)
    k = ctx @ Wk + bk ; v = ctx @ Wv + bv
    dots[h,i,j] = q_h[i] . k_h[j]  + frag_mask[j]   (masked j -> -inf)
    attn = softmax_j(dots) ; out = (attn @ v) @ W_out + b_out

Distribution: 8 cores = 4 batches x 2 head-groups (4 heads each).
Host-side prep: compact context along j by the boolean mask (~50% kept),
transpose x/context to [dim, tokens] layout, slice weights per head group.

Device per core:
  - QKV projections on PE in fp16.  q is pre-scaled by C1 = 128*log2(e)
    on the host so the score matmul directly produces s' = C1*(q.k).
  - S^T = K.QT per head: K=32 row-tiled fp16 matmuls.
  - exp on ACT with per-partition bias=frag and scale=1/C1 (exact).
    A hybrid path exists (n_dve>0): DVE-Schraudolph bit-trick exp --
    one tensor_scalar add of am2 = C1*frag + C2 with int16 output whose
    bits ARE the bf16 pattern of ~exp(s+frag) (~+-3% ripple, end-to-end
    softmax error ~5e-3 vs the 2e-2 gate; HW-validated).  It is OFF in
    BEST_CFG: the slower DVE consumer in the 2-buffer PSUM rotation
    delayed S-tile reuse and measured net-slower than ACT-only.
  - input DMAs are consolidated (dma_merge) and spread across the three
    DGE queues (sync/scalar HWDGE + gpsimd SWDGE, dma_split): the bulk
    context load no longer serializes on one queue (-25us makespan).
  - P @ V and softmax denominators via column-tiled bf16 matmuls
    accumulated in PSUM across j-tiles (denominator uses an all-ones
    [128,32] stationary so it lands broadcast across each head's 32
    partitions).
  - normalize with DVE reciprocal+mul, project with W_out (v-bias and
    b_out are folded into the host-side output assembly, exactly).
"""

import numpy as np
import ml_dtypes

import concourse.bass as bass  # noqa: F401
import concourse.mybir as mybir
import concourse.tile as tile
import concourse.bacc as bacc
from concourse.bass_utils import run_bass_kernel_spmd

F32 = mybir.dt.float32
F32R = mybir.dt.float32r
F16 = mybir.dt.float16
BF16 = mybir.dt.bfloat16
I16 = mybir.dt.int16
AF = mybir.ActivationFunctionType

B = 4
N_Q = 1024          # queries per batch
DIM = 256           # model dim
D_HEAD = 32
HPC = 4             # heads per core
HD = 128            # HPC * D_HEAD: head-group width
NEG = -60000.0      # additive mask for dropped/padded j (exp -> exactly 0)

LOG2E = 1.4426950408889634
C1 = 128.0 * LOG2E              # Schraudolph pre-scale (folded into Wq)
C_SHIFT = 0.0430                # centers the linear-mantissa ripple
C2 = 128.0 * (127.0 - C_SHIFT)  # Schraudolph exponent-bias magic

# DVE share of the exp work: tiles (jt, h) with dve_pick(jt, h) True go to
# the Schraudolph path, the rest to ACT.  Balanced so ACT (853ns/tile) and
# DVE (~1100ns/tile + ~9us misc) finish together.
def _dve_sched(mjt, dve_jt_max, n_dve):
    """Spread n_dve tiles evenly over the eligible (jt, h) grid, late
    heads first within each jt so ACT starts each tile's exp chain."""
    njt = min(dve_jt_max, mjt)
    if njt <= 0 or n_dve <= 0:
        return set()
    n_dve = min(n_dve, njt * HPC)
    picked = set()
    per = n_dve / njt
    acc = 0.0
    total = 0
    for jt in range(njt):
        acc += per
        k = int(round(acc)) - total
        total += k
        for h in (3, 2, 1, 0)[:k]:
            picked.add((jt, h))
    return picked


_cache: dict = {}
last_results = None  # test.py introspection

# Final configuration: exact ACT exp only (the DVE-Schraudolph hybrid is
# correct but measured slightly slower end-to-end -- the slower DVE
# consumer in the 2-buffer PSUM rotation delays S-tile reuse), DMA loads
# split across the three DGE queues and consolidated into few transfers.
BEST_CFG = dict(n_dve=0, dma_split=1, dma_merge=1, pe_warm=32)


def _build(mjt: int, reps: int = 1, debug: bool = False,
           n_dve: int = 26, dve_jt_max: int = 10 ** 9,
           strip_exp: int = 0, strip_pvl: int = 0, dma_split: int = 0,
           ps512: int = 0, pv_delay: int = 1, kv_ahead: int = 0,
           dma_merge: int = 0, head_split: int = 0, pe_warm: int = 0):
    """Build + compile the per-core Bass program for mjt j-tiles of 128.

    reps>1 replicates the whole body serially (bench slope timing only).
    debug adds intermediate-dump outputs (diagnosis only).
    """
    mp = mjt * 128
    dve_set = _dve_sched(mjt, dve_jt_max, n_dve)
    nc = bacc.Bacc("TRN2", target_bir_lowering=False, debug=False)

    if dma_merge:
        # consolidated params: fewer dma_starts -> fewer ~2us HBM-receipt
        # completion latencies on the queues
        d_xT = nc.declare_dram_parameter("xT", [2, 128, N_Q], F16, isOutput=False)
        d_cT = nc.declare_dram_parameter("cT", [2, 128, mp], F16, isOutput=False)
        d_wqkv = nc.declare_dram_parameter("wqkv", [2, 128, 3 * HD], F16, isOutput=False)
        d_wo = nc.declare_dram_parameter("wo_ones", [128, DIM + D_HEAD], F16, isOutput=False)
        d_bias = nc.declare_dram_parameter("bias", [128, 2 * mjt + 2], F32, isOutput=False)
    else:
        d_xT = nc.declare_dram_parameter("xT", [2, 128, N_Q], F16, isOutput=False)
        d_cT = nc.declare_dram_parameter("cT", [2, 128, mp], F16, isOutput=False)
        d_wq = nc.declare_dram_parameter("wq", [2, 128, HD], F16, isOutput=False)
        d_wk = nc.declare_dram_parameter("wk", [2, 128, HD], F16, isOutput=False)
        d_wv = nc.declare_dram_parameter("wv", [2, 128, HD], F16, isOutput=False)
        d_wo = nc.declare_dram_parameter("wo", [128, DIM], F16, isOutput=False)
        d_bq = nc.declare_dram_parameter("bq", [128, 1], F32, isOutput=False)
        d_bk = nc.declare_dram_parameter("bk", [128, 1], F32, isOutput=False)
        d_am = nc.declare_dram_parameter("amask", [128, mjt], F32, isOutput=False)
        d_am2 = nc.declare_dram_parameter("amask2", [128, mjt], F32, isOutput=False)
        d_ones = nc.declare_dram_parameter("ones", [128, D_HEAD], BF16, isOutput=False)
    d_out = nc.declare_dram_parameter("outT", [2, 128, N_Q], F16, isOutput=True)
    if debug:
        d_dbg = {
            nm: nc.declare_dram_parameter(f"dbg_{nm}", [128, width], F32, isOutput=True)
            for nm, width in [("l", N_Q), ("pv", N_Q), ("q", N_Q), ("k", mp), ("at", N_Q)]
        }

    with tile.TileContext(nc) as tc:
        with (
            tc.tile_pool(name="pin", bufs=1) as pin,
            tc.tile_pool(name="pwork", bufs=1) as pwork,
            tc.tile_pool(name="pe", bufs=6 + 4 * pv_delay) as pe_pool,
            tc.tile_pool(name="ps_s", bufs=(4 if ps512 else 2), space="PSUM") as ps_s,
            tc.tile_pool(name="ps_acc", bufs=1, space="PSUM") as ps_acc,
        ):
          for _rep in range(reps):
            # ---- loads (Q-projection deps first: they gate the PE ramp) ----
            xT = [pin.tile([128, N_Q], F16, tag=f"xT{i}", name=f"xT{i}") for i in range(2)]
            wq = [pin.tile([128, HD], F16, tag=f"wq{i}", name=f"wq{i}") for i in range(2)]
            wk = [pin.tile([128, HD], F16, tag=f"wk{i}", name=f"wk{i}") for i in range(2)]
            wv = [pin.tile([128, HD], F16, tag=f"wv{i}", name=f"wv{i}") for i in range(2)]
            # critical chain (gates qT/kT/S(jt0)/first exp) on the sync
            # HWDGE ring; bulk loads go via gpsimd SWDGE so the two DMA
            # paths run in parallel and the first exp starts sooner.
            bq = pin.tile([128, 1], F32)
            if dma_merge:
                # 9 input DMAs total across 3 queues
                wqkv = pin.tile([128, 2 * 3 * HD], F16, tag="wqkv")
                nc.sync.dma_start(wqkv[:, 0:3 * HD], d_wqkv[0])
                nc.sync.dma_start(wqkv[:, 3 * HD:], d_wqkv[1])
                for ct in range(2):
                    wq[ct] = wqkv[:, ct * 3 * HD + 0:ct * 3 * HD + HD]
                    wk[ct] = wqkv[:, ct * 3 * HD + HD:ct * 3 * HD + 2 * HD]
                    wv[ct] = wqkv[:, ct * 3 * HD + 2 * HD:ct * 3 * HD + 3 * HD]
                if head_split:
                    # per-(ct, i-half) quadrants on both HWDGE queues so the
                    # Q-projection's first half starts at half the load time
                    nc.sync.dma_start(xT[0][:, 0:512], d_xT[0][:, 0:512])
                    nc.scalar.dma_start(xT[1][:, 0:512], d_xT[1][:, 0:512])
                    nc.sync.dma_start(xT[0][:, 512:1024], d_xT[0][:, 512:1024])
                    nc.scalar.dma_start(xT[1][:, 512:1024], d_xT[1][:, 512:1024])
                else:
                    nc.sync.dma_start(xT[0][:], d_xT[0])
                    nc.scalar.dma_start(xT[1][:], d_xT[1])
                bias_t = pin.tile([128, 2 * mjt + 2], F32, tag="bias_t")
                nc.sync.dma_start(bias_t[:], d_bias[:])
                am = bias_t[:, 0:mjt]
                am2 = bias_t[:, mjt:2 * mjt]
                bq = bias_t[:, 2 * mjt:2 * mjt + 1]
                bk = bias_t[:, 2 * mjt + 1:2 * mjt + 2]
                cT = [pin.tile([128, mp], F16, tag=f"cT{i}", name=f"cTm{i}") for i in range(2)]
                nc.sync.dma_start(cT[0][:, 0:512], d_cT[0][:, 0:512])
                nc.gpsimd.dma_start(cT[1][:, 0:512], d_cT[1][:, 0:512])
                nc.gpsimd.dma_start(cT[0][:, 512:mp], d_cT[0][:, 512:mp])
                nc.scalar.dma_start(cT[1][:, 512:mp], d_cT[1][:, 512:mp])
                wo_t = pin.tile([128, DIM + D_HEAD], F16, tag="wo_t")
                nc.gpsimd.dma_start(wo_t[:], d_wo[:])
                wo = wo_t[:, 0:DIM]
                ones = wo_t[:, DIM:DIM + D_HEAD].bitcast(BF16)
            elif dma_split:
                # parallelize the head loads across all three DGE paths so
                # the first exp isn't gated on one queue draining ~1MB
                for ct in range(2):
                    nc.sync.dma_start(wq[ct][:], d_wq[ct])
                nc.sync.dma_start(xT[0][:, 0:512], d_xT[0][:, 0:512])
                nc.scalar.dma_start(xT[1][:, 0:512], d_xT[1][:, 0:512])
                nc.sync.dma_start(bq[:], d_bq[:])
                nc.sync.dma_start(xT[0][:, 512:1024], d_xT[0][:, 512:1024])
                nc.scalar.dma_start(xT[1][:, 512:1024], d_xT[1][:, 512:1024])
            else:
                for ct in range(2):
                    nc.sync.dma_start(wq[ct][:], d_wq[ct])
                    nc.sync.dma_start(xT[ct][:, 0:512], d_xT[ct][:, 0:512])
                nc.sync.dma_start(bq[:], d_bq[:])
                for ct in range(2):
                    nc.sync.dma_start(xT[ct][:, 512:1024], d_xT[ct][:, 512:1024])
            if not dma_merge:
              am = pin.tile([128, mjt], F32)
              nc.sync.dma_start(am[:], d_am[:])
              am2 = pin.tile([128, mjt], F32)
              nc.sync.dma_start(am2[:], d_am2[:])
              bk = pin.tile([128, 1], F32)
              nc.sync.dma_start(bk[:], d_bk[:])
              cT = [pin.tile([128, mp], F16, tag=f"cT{i}", name=f"cT{i}") for i in range(2)]
              if dma_split:
                nc.sync.dma_start(wk[0][:], d_wk[0])
                nc.scalar.dma_start(wk[1][:], d_wk[1])
                nc.sync.dma_start(cT[0][:, 0:512], d_cT[0][:, 0:512])
                nc.gpsimd.dma_start(cT[1][:, 0:512], d_cT[1][:, 0:512])
              else:
                for ct in range(2):
                    nc.sync.dma_start(wk[ct][:], d_wk[ct])
                    # first context chunk early: it gates S(jt0) and the ACT ramp
                    nc.sync.dma_start(cT[ct][:, 0:512], d_cT[ct][:, 0:512])
              ones = pin.tile([128, D_HEAD], BF16)
              nc.gpsimd.dma_start(ones[:], d_ones[:])
              for ct in range(2):
                nc.gpsimd.dma_start(wv[ct][:], d_wv[ct])
              if dma_split:
                # spread the bulk cT load across the other DGE paths so no
                # single queue serializes ~1MB of context
                qs = [nc.gpsimd, nc.scalar]
                qi = 0
                for c0 in range(512, mp, 512):
                    c1 = min(c0 + 512, mp)
                    for ct in range(2):
                        qs[qi % len(qs)].dma_start(cT[ct][:, c0:c1], d_cT[ct][:, c0:c1])
                        qi += 1
              else:
                for c0 in range(512, mp, 512):
                    # chunked so early j-tiles unblock before the whole load
                    c1 = min(c0 + 512, mp)
                    for ct in range(2):
                        nc.gpsimd.dma_start(cT[ct][:, c0:c1], d_cT[ct][:, c0:c1])
              wo = pin.tile([128, DIM], F16)
              nc.gpsimd.dma_start(wo[:], d_wo[:])

            # ---- persistent SBUF working tensors ----
            qT_hi = pwork.tile([128, N_Q], F16)   # [head*dim, i]  (pre-scaled by C1)
            kT_hi = pwork.tile([128, mp], F16)    # [head*dim, j]
            vnat = pwork.tile([128, mp], BF16)    # [j_local, jt*128 + head*dim]
            attnT = pwork.tile([128, N_Q], F16)
            linv = pwork.tile([128, N_Q], F32)
            outT = [pwork.tile([128, N_Q], F16, tag=f"outT{i}", name=f"outT{i}")
                    for i in range(2)]

            # warm the ACT exp table set during the DMA phase
            warm = pwork.tile([128, 1], F32, tag="warm")
            nc.vector.memset(warm[:], 0.0)
            warm2 = pwork.tile([128, 1], F32, tag="warm2")
            nc.scalar.activation(warm2[:], warm[:], AF.Exp)

            if pe_warm:
                # ~3.4us of dummy matmuls during the load phase flips the
                # HAM clock gate to 8/8 before the real PE work starts
                wps = ps_s.tile([128, 512 if ps512 else N_Q], F32, tag="s", name="wps")
                for i in range(pe_warm):
                    nc.tensor.matmul(wps[:, 0:128], wq[0][:], wq[1][:],
                                     start=True, stop=True)

            # ---- persistent PSUM accumulators (explicitly zeroed) ----
            pv_acc = ps_acc.tile([128, N_Q], F32, tag="pv")
            l_acc = ps_acc.tile([128, N_Q], F32, tag="l")
            nc.vector.memset(pv_acc[:], 0.0)
            nc.vector.memset(l_acc[:], 0.0)

            # ---- Q^T projection: qT = Wq^T @ x^T (+bq) ----
            for ih in range(2):
                ps = ps_s.tile([128, 512 if ps512 else N_Q], F32, tag="s", name="ps_q")
                sl = slice(ih * 512, ih * 512 + 512)
                for ct in range(2):
                    nc.tensor.matmul(
                        ps[:, 0:512], wq[ct][:], xT[ct][:, sl],
                        start=(ct == 0), stop=(ct == 1),
                    )
                nc.vector.tensor_scalar_add(qT_hi[:, sl], ps[:, 0:512], bq[:])

            # ---- main loop over j-tiles (software-pipelined: PV/L of the
            # previous tile are emitted AFTER the current tile's S matmuls,
            # so the next exp is never blocked behind them) ----
            def emit_pv(j0_p, e_prev, last):
                if strip_pvl:
                    return
                for ih in range(2):
                    sl = slice(ih * 512, ih * 512 + 512)
                    for h in range(HPC):
                        nc.tensor.matmul(
                            pv_acc[32 * h:32 * h + 32, sl],
                            vnat[:, j0_p + 32 * h:j0_p + 32 * h + 32],
                            e_prev[h][:, sl],
                            start=False, stop=(last and h == HPC - 1),
                            tile_position=(0, 32 * h),
                            skip_group_check=True,
                        )
                    for h in range(HPC):
                        nc.tensor.matmul(
                            l_acc[32 * h:32 * h + 32, sl],
                            ones[:],
                            e_prev[h][:, sl],
                            start=False, stop=(last and h == HPC - 1),
                            tile_position=(0, 32 * h),
                            skip_group_check=True,
                        )

            def emit_kv(jt):
                # K^T and V projections for j-tile jt; the DVE copy/bias ops
                # are emitted here so they precede later exp ops in the DVE
                # FIFO and never gate the next tile's S matmuls
                j0 = jt * 128
                s_w = 512 if ps512 else N_Q
                ps = ps_s.tile([128, s_w], F32, tag="s", name="ps_k")
                for ct in range(2):
                    nc.tensor.matmul(
                        ps[:, 0:128], wk[ct][:], cT[ct][:, j0:j0 + 128],
                        start=(ct == 0), stop=(ct == 1),
                    )
                nc.vector.tensor_scalar_add(kT_hi[:, j0:j0 + 128], ps[:, 0:128], bk[:])

                psv = ps_s.tile([128, s_w], F32, tag="s", name="psv")
                for ct in range(2):
                    nc.tensor.matmul(
                        psv[:, 0:HD], cT[ct][:, j0:j0 + 128], wv[ct][:],
                        start=(ct == 0), stop=(ct == 1),
                    )
                nc.vector.tensor_copy(vnat[:, j0:j0 + 128], psv[:, 0:HD])

            if kv_ahead:
                emit_kv(0)
            pending = []  # [(j0, e_tiles)] awaiting their PV/L emission
            for jt in range(mjt):
                j0 = jt * 128
                if kv_ahead:
                    if jt + 1 < mjt:
                        emit_kv(jt + 1)
                else:
                    emit_kv(jt)

                # S^T + exp per head (hybrid ACT / DVE-Schraudolph)
                e_tiles = []
                for h in range(HPC):
                    hp = slice(32 * h, 32 * h + 32)
                    e_t = pe_pool.tile([128, N_Q], BF16, tag="e")
                    if ps512:
                        for ih in range(2):
                            sl = slice(ih * 512, ih * 512 + 512)
                            s_ps = ps_s.tile([128, 512], F32, tag="s")
                            nc.tensor.matmul(
                                s_ps[:], kT_hi[hp, j0:j0 + 128], qT_hi[hp, sl],
                                start=True, stop=True,
                                tile_position=(32 * h, 0),
                            )
                            if strip_exp:
                                nc.vector.memset(e_t[:, sl], 0.00390625)
                            elif (jt, h) in dve_set:
                                nc.vector.tensor_scalar(
                                    e_t[:, sl].bitcast(I16), s_ps[:],
                                    am2[:, jt:jt + 1], None,
                                    op0=mybir.AluOpType.add,
                                )
                            else:
                                nc.scalar.activation(
                                    e_t[:, sl], s_ps[:], AF.Exp,
                                    bias=am[:, jt:jt + 1], scale=1.0 / C1,
                                )
                        e_tiles.append(e_t)
                        continue
                    s_ps = ps_s.tile([128, N_Q], F32, tag="s")
                    for ih in range(2):
                        sl = slice(ih * 512, ih * 512 + 512)
                        nc.tensor.matmul(
                            s_ps[:, sl],
                            kT_hi[hp, j0:j0 + 128],
                            qT_hi[hp, sl],
                            start=True, stop=True,
                            tile_position=(32 * h, 0),
                        )
                    if strip_exp:
                        nc.vector.memset(e_t[:], 0.00390625)
                    elif (jt, h) in dve_set:
                        nc.vector.tensor_scalar(
                            e_t[:].bitcast(I16), s_ps[:],
                            am2[:, jt:jt + 1], None,
                            op0=mybir.AluOpType.add,
                        )
                    else:
                        nc.scalar.activation(
                            e_t[:], s_ps[:], AF.Exp,
                            bias=am[:, jt:jt + 1], scale=1.0 / C1,
                        )
                    e_tiles.append(e_t)

                # P @ V and row-sums for an EARLIER tile (pv_delay tiles
                # back, so its e inputs are final and the PE never stalls
                # on the exp engines behind queued S matmuls)
                pending.append((j0, e_tiles))
                if len(pending) > pv_delay:
                    pj, pe_t = pending.pop(0)
                    emit_pv(pj, pe_t, last=False)
                if jt == mjt - 1:
                    # flush the rest; the first hides under this tile's exps
                    while pending:
                        pj, pe_t = pending.pop(0)
                        emit_pv(pj, pe_t, last=(not pending))


            # ---- debug dumps ----
            if debug:
                dbg_l_s = pwork.tile([128, N_Q], F32, tag="dbg_l_s")
                nc.vector.tensor_copy(dbg_l_s[:], l_acc[:])
                nc.sync.dma_start(d_dbg["l"][:], dbg_l_s[:])
                dbg_pv_s = pwork.tile([128, N_Q], F32, tag="dbg_pv_s")
                nc.vector.tensor_copy(dbg_pv_s[:], pv_acc[:])
                nc.sync.dma_start(d_dbg["pv"][:], dbg_pv_s[:])
                dbg_q_s = pwork.tile([128, N_Q], F32, tag="dbg_q_s")
                nc.vector.tensor_copy(dbg_q_s[:], qT_hi[:])
                nc.sync.dma_start(d_dbg["q"][:], dbg_q_s[:])
                dbg_k_s = pwork.tile([128, mp], F32, tag="dbg_k_s")
                nc.vector.tensor_copy(dbg_k_s[:], kT_hi[:])
                nc.sync.dma_start(d_dbg["k"][:], dbg_k_s[:])

            # ---- normalize + output projection, per i-half so the PE can
            # start projecting half 0 while the DVE still normalizes half 1
            for ih in range(2):
                sl = slice(ih * 512, ih * 512 + 512)
                nc.vector.reciprocal(linv[:, sl], l_acc[:, sl])
                nc.vector.tensor_tensor(
                    attnT[:, sl], pv_acc[:, sl], linv[:, sl], mybir.AluOpType.mult)
                for dt in range(2):
                    ps = ps_s.tile([128, 512 if ps512 else N_Q], F32, tag="s", name="ps_o")
                    nc.tensor.matmul(
                        ps[:, 0:512], wo[:, dt * 128:dt * 128 + 128], attnT[:, sl],
                        start=True, stop=True,
                    )
                    nc.vector.tensor_copy(outT[dt][:, sl], ps[:, 0:512])
                    nc.sync.dma_start(d_out[dt][:, sl], outT[dt][:, sl])
            if debug:
                nc.sync.dma_start(d_dbg["at"][:], attnT[:])

    nc.compile()
    return nc


def build_in_maps(inputs, keeps, mjt):
    x = np.ascontiguousarray(np.asarray(inputs["x"], dtype=np.float32))
    context = np.ascontiguousarray(np.asarray(inputs["context"], dtype=np.float32))
    frag_mask = np.asarray(inputs["frag_mask"], dtype=np.float32)
    W_qkv = np.ascontiguousarray(np.asarray(inputs["W_qkv"], dtype=np.float32))
    b_qkv = np.asarray(inputs["b_qkv"], dtype=np.float32)
    W_out = np.ascontiguousarray(np.asarray(inputs["W_out"], dtype=np.float32))
    mp = mjt * 128
    ones = np.ones((128, D_HEAD), dtype=ml_dtypes.bfloat16)
    in_maps = []
    for core in range(8):
        b, hh = core % B, core // B
        keep = keeps[b]
        cnt = len(keep)
        cT = np.zeros((DIM, mp), dtype=np.float32)
        cT[:, :cnt] = context[b][keep].T
        amask = np.full((mp,), NEG, dtype=np.float32)
        amask[:cnt] = frag_mask[b][keep]
        amask2 = amask * np.float32(C1) + np.float32(C2)
        hs = slice(hh * HD, (hh + 1) * HD)
        m = {
            "xT": np.ascontiguousarray(x[b].T.reshape(2, 128, N_Q)).astype(np.float16),
            "cT": np.ascontiguousarray(cT.reshape(2, 128, mp)).astype(np.float16),
            "wq": np.ascontiguousarray(W_qkv[:, hs].reshape(2, 128, HD) * np.float32(C1)).astype(np.float16),
            "wk": np.ascontiguousarray(W_qkv[:, 256:512][:, hs].reshape(2, 128, HD)).astype(np.float16),
            "wv": np.ascontiguousarray(W_qkv[:, 512:768][:, hs].reshape(2, 128, HD)).astype(np.float16),
            "wo": np.ascontiguousarray(W_out[hs, :]).astype(np.float16),
            "bq": np.ascontiguousarray(b_qkv[0:256][hs].reshape(128, 1) * np.float32(C1)),
            "bk": np.ascontiguousarray(b_qkv[256:512][hs].reshape(128, 1)),
            "amask": np.ascontiguousarray(amask.reshape(mjt, 128).T),
            "amask2": np.ascontiguousarray(amask2.reshape(mjt, 128).T),
            "ones": ones,
        }
        # merged-layout variants (dma_merge): superset keys are harmless
        m["wqkv"] = np.ascontiguousarray(
            np.concatenate([m["wq"], m["wk"], m["wv"]], axis=2))
        wo_m = np.zeros((128, DIM + D_HEAD), dtype=np.float16)
        wo_m[:, :DIM] = m["wo"]
        wo_m[:, DIM:] = ones.view(np.float16)[:, :D_HEAD]
        m["wo_ones"] = wo_m
        m["bias"] = np.ascontiguousarray(np.concatenate(
            [m["amask"], m["amask2"], m["bq"], m["bk"]], axis=1))
        in_maps.append(m)
    return in_maps


def kernel(x, context, mask, frag_mask, W_qkv, b_qkv, W_out, b_out):
    global last_results
    mask = np.asarray(mask).astype(bool)
    b_out = np.asarray(b_out, dtype=np.float32)

    keeps = [np.nonzero(mask[b])[0] for b in range(B)]
    mjt = max(1, max((len(k) + 127) // 128 for k in keeps))
    # DVE (Schraudolph) tiles must have no padded lanes in any batch
    dve_jt_max = min(len(k) // 128 for k in keeps)

    key = (mjt, 1, False)
    if key not in _cache:
        _cache[key] = _build(mjt, dve_jt_max=dve_jt_max, **BEST_CFG)
    nc = _cache[key]

    inputs = {"x": x, "context": context, "frag_mask": frag_mask,
              "W_qkv": W_qkv, "b_qkv": b_qkv, "W_out": W_out}
    in_maps = build_in_maps(inputs, keeps, mjt)

    res = run_bass_kernel_spmd(nc, in_maps, list(range(8)))
    last_results = res

    out = np.zeros((B, N_Q, DIM), dtype=np.float32)
    for core in range(8):
        b = core % B
        partial = res.results[core]["outT"].astype(np.float32).reshape(DIM, N_Q)
        out[b] += partial.T
    b_qkv = np.asarray(b_qkv, dtype=np.float32)
    out += (b_out + b_qkv[512:768] @ np.asarray(W_out, dtype=np.float32))[None, None, :]
    return out


# revision 39
# speedup vs baseline: 1.4154x; 1.4154x over previous
"""Trainium2 Bass kernel for masked cross-attention (sparse_attention).

Reference computation (per batch b):
    q = x @ Wq + bq                      # [N, hd]   (hd = 8 heads * 32)
    k = ctx @ Wk + bk ; v = ctx @ Wv + bv
    dots[h,i,j] = q_h[i] . k_h[j]  + frag_mask[j]   (masked j -> -inf)
    attn = softmax_j(dots) ; out = (attn @ v) @ W_out + b_out

Distribution: 8 cores = 4 batches x 2 head-groups (4 heads each).
Host-side prep: compact context along j by the boolean mask (~50% kept),
transpose x/context to [dim, tokens] layout, slice weights per head group.

Device per core:
  - QKV projections on PE in fp16.  q is pre-scaled by C1 = 128*log2(e)
    on the host so the score matmul directly produces s' = C1*(q.k).
  - S^T = K.QT per head: K=32 row-tiled fp16 matmuls.
  - exp on ACT with per-partition bias=frag and scale=1/C1 (exact).
    A hybrid path exists (n_dve>0): DVE-Schraudolph bit-trick exp --
    one tensor_scalar add of am2 = C1*frag + C2 with int16 output whose
    bits ARE the bf16 pattern of ~exp(s+frag) (~+-3% ripple, end-to-end
    softmax error ~5e-3 vs the 2e-2 gate; HW-validated).  It is OFF in
    BEST_CFG: the slower DVE consumer in the 2-buffer PSUM rotation
    delayed S-tile reuse and measured net-slower than ACT-only.
  - input DMAs are consolidated (dma_merge) and spread across the three
    DGE queues (sync/scalar HWDGE + gpsimd SWDGE, dma_split): the bulk
    context load no longer serializes on one queue (-25us makespan).
  - P @ V and softmax denominators via column-tiled bf16 matmuls
    accumulated in PSUM across j-tiles (denominator uses an all-ones
    [128,32] stationary so it lands broadcast across each head's 32
    partitions).
  - normalize with DVE reciprocal+mul, project with W_out (v-bias and
    b_out are folded into the host-side output assembly, exactly).
"""

import numpy as np
import ml_dtypes

import concourse.bass as bass  # noqa: F401
import concourse.mybir as mybir
import concourse.tile as tile
import concourse.bacc as bacc
from concourse.bass_utils import run_bass_kernel_spmd

F32 = mybir.dt.float32
F32R = mybir.dt.float32r
F16 = mybir.dt.float16
BF16 = mybir.dt.bfloat16
I16 = mybir.dt.int16
AF = mybir.ActivationFunctionType

B = 4
N_Q = 1024          # queries per batch
DIM = 256           # model dim
D_HEAD = 32
HPC = 4             # heads per core
HD = 128            # HPC * D_HEAD: head-group width
NEG = -60000.0      # additive mask for dropped/padded j (exp -> exactly 0)

LOG2E = 1.4426950408889634
C1 = 128.0 * LOG2E              # Schraudolph pre-scale (folded into Wq)
C_SHIFT = 0.0430                # centers the linear-mantissa ripple
C2 = 128.0 * (127.0 - C_SHIFT)  # Schraudolph exponent-bias magic

# DVE share of the exp work: tiles (jt, h) with dve_pick(jt, h) True go to
# the Schraudolph path, the rest to ACT.  Balanced so ACT (853ns/tile) and
# DVE (~1100ns/tile + ~9us misc) finish together.
def _dve_sched(mjt, dve_jt_max, n_dve):
    """Spread n_dve tiles evenly over the eligible (jt, h) grid, late
    heads first within each jt so ACT starts each tile's exp chain."""
    njt = min(dve_jt_max, mjt)
    if njt <= 0 or n_dve <= 0:
        return set()
    n_dve = min(n_dve, njt * HPC)
    picked = set()
    per = n_dve / njt
    acc = 0.0
    total = 0
    for jt in range(njt):
        acc += per
        k = int(round(acc)) - total
        total += k
        for h in (3, 2, 1, 0)[:k]:
            picked.add((jt, h))
    return picked


_cache: dict = {}
last_results = None  # test.py introspection

# Final configuration: exact ACT exp only (the DVE-Schraudolph hybrid is
# correct but measured slightly slower end-to-end -- the slower DVE
# consumer in the 2-buffer PSUM rotation delays S-tile reuse), DMA loads
# split across the three DGE queues and consolidated into few transfers.
BEST_CFG = dict(n_dve=0, dma_split=1, dma_merge=1, pe_warm=32)


def _build(mjt: int, reps: int = 1, debug: bool = False,
           n_dve: int = 26, dve_jt_max: int = 10 ** 9,
           strip_exp: int = 0, strip_pvl: int = 0, dma_split: int = 0,
           ps512: int = 0, pv_delay: int = 1, kv_ahead: int = 0,
           dma_merge: int = 0, head_split: int = 0, pe_warm: int = 0,
           k_batch: int = 0):
    """Build + compile the per-core Bass program for mjt j-tiles of 128.

    reps>1 replicates the whole body serially (bench slope timing only).
    debug adds intermediate-dump outputs (diagnosis only).
    """
    mp = mjt * 128
    dve_set = _dve_sched(mjt, dve_jt_max, n_dve)
    nc = bacc.Bacc("TRN2", target_bir_lowering=False, debug=False)

    if dma_merge:
        # consolidated params: fewer dma_starts -> fewer ~2us HBM-receipt
        # completion latencies on the queues
        d_xT = nc.declare_dram_parameter("xT", [2, 128, N_Q], F16, isOutput=False)
        d_cT = nc.declare_dram_parameter("cT", [2, 128, mp], F16, isOutput=False)
        d_wqkv = nc.declare_dram_parameter("wqkv", [2, 128, 3 * HD], F16, isOutput=False)
        d_wo = nc.declare_dram_parameter("wo_ones", [128, DIM + D_HEAD], F16, isOutput=False)
        d_bias = nc.declare_dram_parameter("bias", [128, 2 * mjt + 2], F32, isOutput=False)
    else:
        d_xT = nc.declare_dram_parameter("xT", [2, 128, N_Q], F16, isOutput=False)
        d_cT = nc.declare_dram_parameter("cT", [2, 128, mp], F16, isOutput=False)
        d_wq = nc.declare_dram_parameter("wq", [2, 128, HD], F16, isOutput=False)
        d_wk = nc.declare_dram_parameter("wk", [2, 128, HD], F16, isOutput=False)
        d_wv = nc.declare_dram_parameter("wv", [2, 128, HD], F16, isOutput=False)
        d_wo = nc.declare_dram_parameter("wo", [128, DIM], F16, isOutput=False)
        d_bq = nc.declare_dram_parameter("bq", [128, 1], F32, isOutput=False)
        d_bk = nc.declare_dram_parameter("bk", [128, 1], F32, isOutput=False)
        d_am = nc.declare_dram_parameter("amask", [128, mjt], F32, isOutput=False)
        d_am2 = nc.declare_dram_parameter("amask2", [128, mjt], F32, isOutput=False)
        d_ones = nc.declare_dram_parameter("ones", [128, D_HEAD], BF16, isOutput=False)
    d_out = nc.declare_dram_parameter("outT", [2, 128, N_Q], F16, isOutput=True)
    if debug:
        d_dbg = {
            nm: nc.declare_dram_parameter(f"dbg_{nm}", [128, width], F32, isOutput=True)
            for nm, width in [("l", N_Q), ("pv", N_Q), ("q", N_Q), ("k", mp), ("at", N_Q)]
        }

    with tile.TileContext(nc) as tc:
        with (
            tc.tile_pool(name="pin", bufs=1) as pin,
            tc.tile_pool(name="pwork", bufs=1) as pwork,
            tc.tile_pool(name="pe", bufs=6 + 4 * pv_delay) as pe_pool,
            tc.tile_pool(name="ps_s", bufs=(4 if ps512 else 2), space="PSUM") as ps_s,
            tc.tile_pool(name="ps_acc", bufs=1, space="PSUM") as ps_acc,
        ):
          for _rep in range(reps):
            # ---- loads (Q-projection deps first: they gate the PE ramp) ----
            xT = [pin.tile([128, N_Q], F16, tag=f"xT{i}", name=f"xT{i}") for i in range(2)]
            wq = [pin.tile([128, HD], F16, tag=f"wq{i}", name=f"wq{i}") for i in range(2)]
            wk = [pin.tile([128, HD], F16, tag=f"wk{i}", name=f"wk{i}") for i in range(2)]
            wv = [pin.tile([128, HD], F16, tag=f"wv{i}", name=f"wv{i}") for i in range(2)]
            # critical chain (gates qT/kT/S(jt0)/first exp) on the sync
            # HWDGE ring; bulk loads go via gpsimd SWDGE so the two DMA
            # paths run in parallel and the first exp starts sooner.
            bq = pin.tile([128, 1], F32)
            if dma_merge:
                # 9 input DMAs total across 3 queues
                wqkv = pin.tile([128, 2 * 3 * HD], F16, tag="wqkv")
                nc.sync.dma_start(wqkv[:, 0:3 * HD], d_wqkv[0])
                nc.sync.dma_start(wqkv[:, 3 * HD:], d_wqkv[1])
                for ct in range(2):
                    wq[ct] = wqkv[:, ct * 3 * HD + 0:ct * 3 * HD + HD]
                    wk[ct] = wqkv[:, ct * 3 * HD + HD:ct * 3 * HD + 2 * HD]
                    wv[ct] = wqkv[:, ct * 3 * HD + 2 * HD:ct * 3 * HD + 3 * HD]
                if head_split:
                    # per-(ct, i-half) quadrants on both HWDGE queues so the
                    # Q-projection's first half starts at half the load time
                    nc.sync.dma_start(xT[0][:, 0:512], d_xT[0][:, 0:512])
                    nc.scalar.dma_start(xT[1][:, 0:512], d_xT[1][:, 0:512])
                    nc.sync.dma_start(xT[0][:, 512:1024], d_xT[0][:, 512:1024])
                    nc.scalar.dma_start(xT[1][:, 512:1024], d_xT[1][:, 512:1024])
                else:
                    nc.sync.dma_start(xT[0][:], d_xT[0])
                    nc.scalar.dma_start(xT[1][:], d_xT[1])
                bias_t = pin.tile([128, 2 * mjt + 2], F32, tag="bias_t")
                nc.sync.dma_start(bias_t[:], d_bias[:])
                am = bias_t[:, 0:mjt]
                am2 = bias_t[:, mjt:2 * mjt]
                bq = bias_t[:, 2 * mjt:2 * mjt + 1]
                bk = bias_t[:, 2 * mjt + 1:2 * mjt + 2]
                cT = [pin.tile([128, mp], F16, tag=f"cT{i}", name=f"cTm{i}") for i in range(2)]
                nc.sync.dma_start(cT[0][:, 0:512], d_cT[0][:, 0:512])
                nc.gpsimd.dma_start(cT[1][:, 0:512], d_cT[1][:, 0:512])
                nc.gpsimd.dma_start(cT[0][:, 512:mp], d_cT[0][:, 512:mp])
                nc.scalar.dma_start(cT[1][:, 512:mp], d_cT[1][:, 512:mp])
                wo_t = pin.tile([128, DIM + D_HEAD], F16, tag="wo_t")
                nc.gpsimd.dma_start(wo_t[:], d_wo[:])
                wo = wo_t[:, 0:DIM]
                ones = wo_t[:, DIM:DIM + D_HEAD].bitcast(BF16)
            elif dma_split:
                # parallelize the head loads across all three DGE paths so
                # the first exp isn't gated on one queue draining ~1MB
                for ct in range(2):
                    nc.sync.dma_start(wq[ct][:], d_wq[ct])
                nc.sync.dma_start(xT[0][:, 0:512], d_xT[0][:, 0:512])
                nc.scalar.dma_start(xT[1][:, 0:512], d_xT[1][:, 0:512])
                nc.sync.dma_start(bq[:], d_bq[:])
                nc.sync.dma_start(xT[0][:, 512:1024], d_xT[0][:, 512:1024])
                nc.scalar.dma_start(xT[1][:, 512:1024], d_xT[1][:, 512:1024])
            else:
                for ct in range(2):
                    nc.sync.dma_start(wq[ct][:], d_wq[ct])
                    nc.sync.dma_start(xT[ct][:, 0:512], d_xT[ct][:, 0:512])
                nc.sync.dma_start(bq[:], d_bq[:])
                for ct in range(2):
                    nc.sync.dma_start(xT[ct][:, 512:1024], d_xT[ct][:, 512:1024])
            if not dma_merge:
              am = pin.tile([128, mjt], F32)
              nc.sync.dma_start(am[:], d_am[:])
              am2 = pin.tile([128, mjt], F32)
              nc.sync.dma_start(am2[:], d_am2[:])
              bk = pin.tile([128, 1], F32)
              nc.sync.dma_start(bk[:], d_bk[:])
              cT = [pin.tile([128, mp], F16, tag=f"cT{i}", name=f"cT{i}") for i in range(2)]
              if dma_split:
                nc.sync.dma_start(wk[0][:], d_wk[0])
                nc.scalar.dma_start(wk[1][:], d_wk[1])
                nc.sync.dma_start(cT[0][:, 0:512], d_cT[0][:, 0:512])
                nc.gpsimd.dma_start(cT[1][:, 0:512], d_cT[1][:, 0:512])
              else:
                for ct in range(2):
                    nc.sync.dma_start(wk[ct][:], d_wk[ct])
                    # first context chunk early: it gates S(jt0) and the ACT ramp
                    nc.sync.dma_start(cT[ct][:, 0:512], d_cT[ct][:, 0:512])
              ones = pin.tile([128, D_HEAD], BF16)
              nc.gpsimd.dma_start(ones[:], d_ones[:])
              for ct in range(2):
                nc.gpsimd.dma_start(wv[ct][:], d_wv[ct])
              if dma_split:
                # spread the bulk cT load across the other DGE paths so no
                # single queue serializes ~1MB of context
                qs = [nc.gpsimd, nc.scalar]
                qi = 0
                for c0 in range(512, mp, 512):
                    c1 = min(c0 + 512, mp)
                    for ct in range(2):
                        qs[qi % len(qs)].dma_start(cT[ct][:, c0:c1], d_cT[ct][:, c0:c1])
                        qi += 1
              else:
                for c0 in range(512, mp, 512):
                    # chunked so early j-tiles unblock before the whole load
                    c1 = min(c0 + 512, mp)
                    for ct in range(2):
                        nc.gpsimd.dma_start(cT[ct][:, c0:c1], d_cT[ct][:, c0:c1])
              wo = pin.tile([128, DIM], F16)
              nc.gpsimd.dma_start(wo[:], d_wo[:])

            # ---- persistent SBUF working tensors ----
            qT_hi = pwork.tile([128, N_Q], F16)   # [head*dim, i]  (pre-scaled by C1)
            kT_hi = pwork.tile([128, mp], F16)    # [head*dim, j]
            vnat = pwork.tile([128, mp], BF16)    # [j_local, jt*128 + head*dim]
            attnT = pwork.tile([128, N_Q], F16)
            linv = pwork.tile([128, N_Q], F32)
            outT = [pwork.tile([128, N_Q], F16, tag=f"outT{i}", name=f"outT{i}")
                    for i in range(2)]

            # warm the ACT exp table set during the DMA phase
            warm = pwork.tile([128, 1], F32, tag="warm")
            nc.vector.memset(warm[:], 0.0)
            warm2 = pwork.tile([128, 1], F32, tag="warm2")
            nc.scalar.activation(warm2[:], warm[:], AF.Exp)

            if pe_warm:
                # ~3.4us of dummy matmuls during the load phase flips the
                # HAM clock gate to 8/8 before the real PE work starts
                wps = ps_s.tile([128, 512 if ps512 else N_Q], F32, tag="s", name="wps")
                for i in range(pe_warm):
                    nc.tensor.matmul(wps[:, 0:128], wq[0][:], wq[1][:],
                                     start=True, stop=True)

            # ---- persistent PSUM accumulators (explicitly zeroed) ----
            pv_acc = ps_acc.tile([128, N_Q], F32, tag="pv")
            l_acc = ps_acc.tile([128, N_Q], F32, tag="l")
            nc.vector.memset(pv_acc[:], 0.0)
            nc.vector.memset(l_acc[:], 0.0)

            # ---- Q^T projection: qT = Wq^T @ x^T (+bq) ----
            for ih in range(2):
                ps = ps_s.tile([128, 512 if ps512 else N_Q], F32, tag="s", name="ps_q")
                sl = slice(ih * 512, ih * 512 + 512)
                for ct in range(2):
                    nc.tensor.matmul(
                        ps[:, 0:512], wq[ct][:], xT[ct][:, sl],
                        start=(ct == 0), stop=(ct == 1),
                    )
                nc.vector.tensor_scalar_add(qT_hi[:, sl], ps[:, 0:512], bq[:])

            # ---- main loop over j-tiles (software-pipelined: PV/L of the
            # previous tile are emitted AFTER the current tile's S matmuls,
            # so the next exp is never blocked behind them) ----
            def emit_pv(j0_p, e_prev, last):
                if strip_pvl:
                    return
                for ih in range(2):
                    sl = slice(ih * 512, ih * 512 + 512)
                    for h in range(HPC):
                        nc.tensor.matmul(
                            pv_acc[32 * h:32 * h + 32, sl],
                            vnat[:, j0_p + 32 * h:j0_p + 32 * h + 32],
                            e_prev[h][:, sl],
                            start=False, stop=(last and h == HPC - 1),
                            tile_position=(0, 32 * h),
                            skip_group_check=True,
                        )
                    for h in range(HPC):
                        nc.tensor.matmul(
                            l_acc[32 * h:32 * h + 32, sl],
                            ones[:],
                            e_prev[h][:, sl],
                            start=False, stop=(last and h == HPC - 1),
                            tile_position=(0, 32 * h),
                            skip_group_check=True,
                        )

            if k_batch:
                # whole K^T projection as a prologue (512-wide groups):
                # removes the per-tile K->kT-add->S cross-engine chain and
                # shrinks 17 small DVE bias-adds to 5 large ones
                for g0 in range(0, mp, 512):
                    g1 = min(g0 + 512, mp)
                    psk = ps_s.tile([128, 512 if ps512 else N_Q], F32,
                                    tag="s", name="psk")
                    for ct in range(2):
                        nc.tensor.matmul(
                            psk[:, 0:g1 - g0], wk[ct][:], cT[ct][:, g0:g1],
                            start=(ct == 0), stop=(ct == 1),
                        )
                    nc.vector.tensor_scalar_add(
                        kT_hi[:, g0:g1], psk[:, 0:g1 - g0], bk[:])

            def emit_kv(jt):
                # K^T and V projections for j-tile jt; the DVE copy/bias ops
                # are emitted here so they precede later exp ops in the DVE
                # FIFO and never gate the next tile's S matmuls
                j0 = jt * 128
                s_w = 512 if ps512 else N_Q
                if not k_batch:
                    ps = ps_s.tile([128, s_w], F32, tag="s", name="ps_k")
                    for ct in range(2):
                        nc.tensor.matmul(
                            ps[:, 0:128], wk[ct][:], cT[ct][:, j0:j0 + 128],
                            start=(ct == 0), stop=(ct == 1),
                        )
                    nc.vector.tensor_scalar_add(kT_hi[:, j0:j0 + 128], ps[:, 0:128], bk[:])

                psv = ps_s.tile([128, s_w], F32, tag="s", name="psv")
                for ct in range(2):
                    nc.tensor.matmul(
                        psv[:, 0:HD], cT[ct][:, j0:j0 + 128], wv[ct][:],
                        start=(ct == 0), stop=(ct == 1),
                    )
                nc.vector.tensor_copy(vnat[:, j0:j0 + 128], psv[:, 0:HD])

            if kv_ahead:
                emit_kv(0)
            pending = []  # [(j0, e_tiles)] awaiting their PV/L emission
            for jt in range(mjt):
                j0 = jt * 128
                if kv_ahead:
                    if jt + 1 < mjt:
                        emit_kv(jt + 1)
                else:
                    emit_kv(jt)

                # S^T + exp per head (hybrid ACT / DVE-Schraudolph)
                e_tiles = []
                for h in range(HPC):
                    hp = slice(32 * h, 32 * h + 32)
                    e_t = pe_pool.tile([128, N_Q], BF16, tag="e")
                    if ps512:
                        for ih in range(2):
                            sl = slice(ih * 512, ih * 512 + 512)
                            s_ps = ps_s.tile([128, 512], F32, tag="s")
                            nc.tensor.matmul(
                                s_ps[:], kT_hi[hp, j0:j0 + 128], qT_hi[hp, sl],
                                start=True, stop=True,
                                tile_position=(32 * h, 0),
                            )
                            if strip_exp:
                                nc.vector.memset(e_t[:, sl], 0.00390625)
                            elif (jt, h) in dve_set:
                                nc.vector.tensor_scalar(
                                    e_t[:, sl].bitcast(I16), s_ps[:],
                                    am2[:, jt:jt + 1], None,
                                    op0=mybir.AluOpType.add,
                                )
                            else:
                                nc.scalar.activation(
                                    e_t[:, sl], s_ps[:], AF.Exp,
                                    bias=am[:, jt:jt + 1], scale=1.0 / C1,
                                )
                        e_tiles.append(e_t)
                        continue
                    s_ps = ps_s.tile([128, N_Q], F32, tag="s")
                    for ih in range(2):
                        sl = slice(ih * 512, ih * 512 + 512)
                        nc.tensor.matmul(
                            s_ps[:, sl],
                            kT_hi[hp, j0:j0 + 128],
                            qT_hi[hp, sl],
                            start=True, stop=True,
                            tile_position=(32 * h, 0),
                        )
                    if strip_exp:
                        nc.vector.memset(e_t[:], 0.00390625)
                    elif (jt, h) in dve_set:
                        nc.vector.tensor_scalar(
                            e_t[:].bitcast(I16), s_ps[:],
                            am2[:, jt:jt + 1], None,
                            op0=mybir.AluOpType.add,
                        )
                    else:
                        nc.scalar.activation(
                            e_t[:], s_ps[:], AF.Exp,
                            bias=am[:, jt:jt + 1], scale=1.0 / C1,
                        )
                    e_tiles.append(e_t)

                # P @ V and row-sums for an EARLIER tile (pv_delay tiles
                # back, so its e inputs are final and the PE never stalls
                # on the exp engines behind queued S matmuls)
                pending.append((j0, e_tiles))
                if len(pending) > pv_delay:
                    pj, pe_t = pending.pop(0)
                    emit_pv(pj, pe_t, last=False)
                if jt == mjt - 1:
                    # flush the rest; the first hides under this tile's exps
                    while pending:
                        pj, pe_t = pending.pop(0)
                        emit_pv(pj, pe_t, last=(not pending))


            # ---- debug dumps ----
            if debug:
                dbg_l_s = pwork.tile([128, N_Q], F32, tag="dbg_l_s")
                nc.vector.tensor_copy(dbg_l_s[:], l_acc[:])
                nc.sync.dma_start(d_dbg["l"][:], dbg_l_s[:])
                dbg_pv_s = pwork.tile([128, N_Q], F32, tag="dbg_pv_s")
                nc.vector.tensor_copy(dbg_pv_s[:], pv_acc[:])
                nc.sync.dma_start(d_dbg["pv"][:], dbg_pv_s[:])
                dbg_q_s = pwork.tile([128, N_Q], F32, tag="dbg_q_s")
                nc.vector.tensor_copy(dbg_q_s[:], qT_hi[:])
                nc.sync.dma_start(d_dbg["q"][:], dbg_q_s[:])
                dbg_k_s = pwork.tile([128, mp], F32, tag="dbg_k_s")
                nc.vector.tensor_copy(dbg_k_s[:], kT_hi[:])
                nc.sync.dma_start(d_dbg["k"][:], dbg_k_s[:])

            # ---- normalize + output projection, per i-half so the PE can
            # start projecting half 0 while the DVE still normalizes half 1
            for ih in range(2):
                sl = slice(ih * 512, ih * 512 + 512)
                nc.vector.reciprocal(linv[:, sl], l_acc[:, sl])
                nc.vector.tensor_tensor(
                    attnT[:, sl], pv_acc[:, sl], linv[:, sl], mybir.AluOpType.mult)
                for dt in range(2):
                    ps = ps_s.tile([128, 512 if ps512 else N_Q], F32, tag="s", name="ps_o")
                    nc.tensor.matmul(
                        ps[:, 0:512], wo[:, dt * 128:dt * 128 + 128], attnT[:, sl],
                        start=True, stop=True,
                    )
                    nc.vector.tensor_copy(outT[dt][:, sl], ps[:, 0:512])
                    nc.sync.dma_start(d_out[dt][:, sl], outT[dt][:, sl])
            if debug:
                nc.sync.dma_start(d_dbg["at"][:], attnT[:])

    nc.compile()
    return nc


def build_in_maps(inputs, keeps, mjt):
    x = np.ascontiguousarray(np.asarray(inputs["x"], dtype=np.float32))
    context = np.ascontiguousarray(np.asarray(inputs["context"], dtype=np.float32))
    frag_mask = np.asarray(inputs["frag_mask"], dtype=np.float32)
    W_qkv = np.ascontiguousarray(np.asarray(inputs["W_qkv"], dtype=np.float32))
    b_qkv = np.asarray(inputs["b_qkv"], dtype=np.float32)
    W_out = np.ascontiguousarray(np.asarray(inputs["W_out"], dtype=np.float32))
    mp = mjt * 128
    ones = np.ones((128, D_HEAD), dtype=ml_dtypes.bfloat16)
    in_maps = []
    for core in range(8):
        b, hh = core % B, core // B
        keep = keeps[b]
        cnt = len(keep)
        cT = np.zeros((DIM, mp), dtype=np.float32)
        cT[:, :cnt] = context[b][keep].T
        amask = np.full((mp,), NEG, dtype=np.float32)
        amask[:cnt] = frag_mask[b][keep]
        amask2 = amask * np.float32(C1) + np.float32(C2)
        hs = slice(hh * HD, (hh + 1) * HD)
        m = {
            "xT": np.ascontiguousarray(x[b].T.reshape(2, 128, N_Q)).astype(np.float16),
            "cT": np.ascontiguousarray(cT.reshape(2, 128, mp)).astype(np.float16),
            "wq": np.ascontiguousarray(W_qkv[:, hs].reshape(2, 128, HD) * np.float32(C1)).astype(np.float16),
            "wk": np.ascontiguousarray(W_qkv[:, 256:512][:, hs].reshape(2, 128, HD)).astype(np.float16),
            "wv": np.ascontiguousarray(W_qkv[:, 512:768][:, hs].reshape(2, 128, HD)).astype(np.float16),
            "wo": np.ascontiguousarray(W_out[hs, :]).astype(np.float16),
            "bq": np.ascontiguousarray(b_qkv[0:256][hs].reshape(128, 1) * np.float32(C1)),
            "bk": np.ascontiguousarray(b_qkv[256:512][hs].reshape(128, 1)),
            "amask": np.ascontiguousarray(amask.reshape(mjt, 128).T),
            "amask2": np.ascontiguousarray(amask2.reshape(mjt, 128).T),
            "ones": ones,
        }
        # merged-layout variants (dma_merge): superset keys are harmless
        m["wqkv"] = np.ascontiguousarray(
            np.concatenate([m["wq"], m["wk"], m["wv"]], axis=2))
        wo_m = np.zeros((128, DIM + D_HEAD), dtype=np.float16)
        wo_m[:, :DIM] = m["wo"]
        wo_m[:, DIM:] = ones.view(np.float16)[:, :D_HEAD]
        m["wo_ones"] = wo_m
        m["bias"] = np.ascontiguousarray(np.concatenate(
            [m["amask"], m["amask2"], m["bq"], m["bk"]], axis=1))
        in_maps.append(m)
    return in_maps


def kernel(x, context, mask, frag_mask, W_qkv, b_qkv, W_out, b_out):
    global last_results
    mask = np.asarray(mask).astype(bool)
    b_out = np.asarray(b_out, dtype=np.float32)

    keeps = [np.nonzero(mask[b])[0] for b in range(B)]
    mjt = max(1, max((len(k) + 127) // 128 for k in keeps))
    # DVE (Schraudolph) tiles must have no padded lanes in any batch
    dve_jt_max = min(len(k) // 128 for k in keeps)

    key = (mjt, 1, False)
    if key not in _cache:
        _cache[key] = _build(mjt, dve_jt_max=dve_jt_max, **BEST_CFG)
    nc = _cache[key]

    inputs = {"x": x, "context": context, "frag_mask": frag_mask,
              "W_qkv": W_qkv, "b_qkv": b_qkv, "W_out": W_out}
    in_maps = build_in_maps(inputs, keeps, mjt)

    res = run_bass_kernel_spmd(nc, in_maps, list(range(8)))
    last_results = res

    out = np.zeros((B, N_Q, DIM), dtype=np.float32)
    for core in range(8):
        b = core % B
        partial = res.results[core]["outT"].astype(np.float32).reshape(DIM, N_Q)
        out[b] += partial.T
    b_qkv = np.asarray(b_qkv, dtype=np.float32)
    out += (b_out + b_qkv[512:768] @ np.asarray(W_out, dtype=np.float32))[None, None, :]
    return out


# revision 43
# speedup vs baseline: 1.7483x; 1.2352x over previous
"""Trainium2 Bass kernel for masked cross-attention (sparse_attention).

Reference computation (per batch b):
    q = x @ Wq + bq                      # [N, hd]   (hd = 8 heads * 32)
    k = ctx @ Wk + bk ; v = ctx @ Wv + bv
    dots[h,i,j] = q_h[i] . k_h[j]  + frag_mask[j]   (masked j -> -inf)
    attn = softmax_j(dots) ; out = (attn @ v) @ W_out + b_out

Distribution: 8 cores = 4 batches x 2 head-groups (4 heads each).
Host-side prep: compact context along j by the boolean mask (~50% kept),
transpose x/context to [dim, tokens] layout, slice weights per head group.

Device per core:
  - QKV projections on PE in fp16.  q is pre-scaled by C1 = 128*log2(e)
    on the host so the score matmul directly produces s' = C1*(q.k).
  - S^T = K.QT per head: K=32 row-tiled fp16 matmuls.
  - exp on ACT with per-partition bias=frag and scale=1/C1 (exact).
    A hybrid path exists (n_dve>0): DVE-Schraudolph bit-trick exp --
    one tensor_scalar add of am2 = C1*frag + C2 with int16 output whose
    bits ARE the bf16 pattern of ~exp(s+frag) (~+-3% ripple, end-to-end
    softmax error ~5e-3 vs the 2e-2 gate; HW-validated).  It is OFF in
    BEST_CFG: the slower DVE consumer in the 2-buffer PSUM rotation
    delayed S-tile reuse and measured net-slower than ACT-only.
  - input DMAs are consolidated (dma_merge) and spread across the three
    DGE queues (sync/scalar HWDGE + gpsimd SWDGE, dma_split): the bulk
    context load no longer serializes on one queue (-25us makespan).
  - P @ V and softmax denominators via column-tiled bf16 matmuls
    accumulated in PSUM across j-tiles (denominator uses an all-ones
    [128,32] stationary so it lands broadcast across each head's 32
    partitions).
  - normalize with DVE reciprocal+mul, project with W_out (v-bias and
    b_out are folded into the host-side output assembly, exactly).
"""

import numpy as np
import ml_dtypes

import concourse.bass as bass  # noqa: F401
import concourse.mybir as mybir
import concourse.tile as tile
import concourse.bacc as bacc
from concourse.bass_utils import run_bass_kernel_spmd

F32 = mybir.dt.float32
F32R = mybir.dt.float32r
F16 = mybir.dt.float16
BF16 = mybir.dt.bfloat16
I16 = mybir.dt.int16
AF = mybir.ActivationFunctionType

B = 4
N_Q = 1024          # queries per batch
DIM = 256           # model dim
D_HEAD = 32
HPC = 4             # heads per core
HD = 128            # HPC * D_HEAD: head-group width
NEG = -60000.0      # additive mask for dropped/padded j (exp -> exactly 0)

LOG2E = 1.4426950408889634
C1 = 128.0 * LOG2E              # Schraudolph pre-scale (folded into Wq)
C_SHIFT = 0.0430                # centers the linear-mantissa ripple
C2 = 128.0 * (127.0 - C_SHIFT)  # Schraudolph exponent-bias magic

# DVE share of the exp work: tiles (jt, h) with dve_pick(jt, h) True go to
# the Schraudolph path, the rest to ACT.  Balanced so ACT (853ns/tile) and
# DVE (~1100ns/tile + ~9us misc) finish together.
def _dve_sched(mjt, dve_jt_max, n_dve):
    """Spread n_dve tiles evenly over the eligible (jt, h) grid, late
    heads first within each jt so ACT starts each tile's exp chain."""
    njt = min(dve_jt_max, mjt)
    if njt <= 0 or n_dve <= 0:
        return set()
    n_dve = min(n_dve, njt * HPC)
    picked = set()
    per = n_dve / njt
    acc = 0.0
    total = 0
    for jt in range(njt):
        acc += per
        k = int(round(acc)) - total
        total += k
        for h in (3, 2, 1, 0)[:k]:
            picked.add((jt, h))
    return picked


_cache: dict = {}
last_results = None  # test.py introspection

# Final configuration: exact ACT exp only (the DVE-Schraudolph hybrid is
# correct but measured slightly slower end-to-end -- the slower DVE
# consumer in the 2-buffer PSUM rotation delays S-tile reuse), DMA loads
# split across the three DGE queues and consolidated into few transfers.
BEST_CFG = dict(n_dve=0, dma_split=1, dma_merge=1, pe_warm=32)


def _build(mjt: int, reps: int = 1, debug: bool = False,
           n_dve: int = 26, dve_jt_max: int = 10 ** 9,
           strip_exp: int = 0, strip_pvl: int = 0, dma_split: int = 0,
           ps512: int = 0, pv_delay: int = 1, kv_ahead: int = 0,
           dma_merge: int = 0, head_split: int = 0, pe_warm: int = 0,
           k_batch: int = 0, c_chunk: int = 0, tail_act: int = 0):
    """Build + compile the per-core Bass program for mjt j-tiles of 128.

    reps>1 replicates the whole body serially (bench slope timing only).
    debug adds intermediate-dump outputs (diagnosis only).
    """
    mp = mjt * 128
    dve_set = _dve_sched(mjt, dve_jt_max, n_dve)
    nc = bacc.Bacc("TRN2", target_bir_lowering=False, debug=False)

    if dma_merge:
        # consolidated params: fewer dma_starts -> fewer ~2us HBM-receipt
        # completion latencies on the queues
        d_xT = nc.declare_dram_parameter("xT", [2, 128, N_Q], F16, isOutput=False)
        d_cT = nc.declare_dram_parameter("cT", [2, 128, mp], F16, isOutput=False)
        d_wqkv = nc.declare_dram_parameter("wqkv", [2, 128, 3 * HD], F16, isOutput=False)
        d_wo = nc.declare_dram_parameter("wo_ones", [128, DIM + D_HEAD], F16, isOutput=False)
        d_bias = nc.declare_dram_parameter("bias", [128, 2 * mjt + 2], F32, isOutput=False)
    else:
        d_xT = nc.declare_dram_parameter("xT", [2, 128, N_Q], F16, isOutput=False)
        d_cT = nc.declare_dram_parameter("cT", [2, 128, mp], F16, isOutput=False)
        d_wq = nc.declare_dram_parameter("wq", [2, 128, HD], F16, isOutput=False)
        d_wk = nc.declare_dram_parameter("wk", [2, 128, HD], F16, isOutput=False)
        d_wv = nc.declare_dram_parameter("wv", [2, 128, HD], F16, isOutput=False)
        d_wo = nc.declare_dram_parameter("wo", [128, DIM], F16, isOutput=False)
        d_bq = nc.declare_dram_parameter("bq", [128, 1], F32, isOutput=False)
        d_bk = nc.declare_dram_parameter("bk", [128, 1], F32, isOutput=False)
        d_am = nc.declare_dram_parameter("amask", [128, mjt], F32, isOutput=False)
        d_am2 = nc.declare_dram_parameter("amask2", [128, mjt], F32, isOutput=False)
        d_ones = nc.declare_dram_parameter("ones", [128, D_HEAD], BF16, isOutput=False)
    d_out = nc.declare_dram_parameter("outT", [2, 128, N_Q], F16, isOutput=True)
    if debug:
        d_dbg = {
            nm: nc.declare_dram_parameter(f"dbg_{nm}", [128, width], F32, isOutput=True)
            for nm, width in [("l", N_Q), ("pv", N_Q), ("q", N_Q), ("k", mp), ("at", N_Q)]
        }

    with tile.TileContext(nc) as tc:
        with (
            tc.tile_pool(name="pin", bufs=1) as pin,
            tc.tile_pool(name="pwork", bufs=1) as pwork,
            tc.tile_pool(name="pe", bufs=6 + 4 * pv_delay) as pe_pool,
            tc.tile_pool(name="ps_s", bufs=(4 if ps512 else 2), space="PSUM") as ps_s,
            tc.tile_pool(name="ps_acc", bufs=1, space="PSUM") as ps_acc,
        ):
          for _rep in range(reps):
            # ---- loads (Q-projection deps first: they gate the PE ramp) ----
            xT = [pin.tile([128, N_Q], F16, tag=f"xT{i}", name=f"xT{i}") for i in range(2)]
            wq = [pin.tile([128, HD], F16, tag=f"wq{i}", name=f"wq{i}") for i in range(2)]
            wk = [pin.tile([128, HD], F16, tag=f"wk{i}", name=f"wk{i}") for i in range(2)]
            wv = [pin.tile([128, HD], F16, tag=f"wv{i}", name=f"wv{i}") for i in range(2)]
            # critical chain (gates qT/kT/S(jt0)/first exp) on the sync
            # HWDGE ring; bulk loads go via gpsimd SWDGE so the two DMA
            # paths run in parallel and the first exp starts sooner.
            bq = pin.tile([128, 1], F32)
            if dma_merge:
                # 9 input DMAs total across 3 queues
                wqkv = pin.tile([128, 2 * 3 * HD], F16, tag="wqkv")
                nc.sync.dma_start(wqkv[:, 0:3 * HD], d_wqkv[0])
                nc.sync.dma_start(wqkv[:, 3 * HD:], d_wqkv[1])
                for ct in range(2):
                    wq[ct] = wqkv[:, ct * 3 * HD + 0:ct * 3 * HD + HD]
                    wk[ct] = wqkv[:, ct * 3 * HD + HD:ct * 3 * HD + 2 * HD]
                    wv[ct] = wqkv[:, ct * 3 * HD + 2 * HD:ct * 3 * HD + 3 * HD]
                if head_split:
                    # per-(ct, i-half) quadrants on both HWDGE queues so the
                    # Q-projection's first half starts at half the load time
                    nc.sync.dma_start(xT[0][:, 0:512], d_xT[0][:, 0:512])
                    nc.scalar.dma_start(xT[1][:, 0:512], d_xT[1][:, 0:512])
                    nc.sync.dma_start(xT[0][:, 512:1024], d_xT[0][:, 512:1024])
                    nc.scalar.dma_start(xT[1][:, 512:1024], d_xT[1][:, 512:1024])
                else:
                    nc.sync.dma_start(xT[0][:], d_xT[0])
                    nc.scalar.dma_start(xT[1][:], d_xT[1])
                bias_t = pin.tile([128, 2 * mjt + 2], F32, tag="bias_t")
                nc.sync.dma_start(bias_t[:], d_bias[:])
                am = bias_t[:, 0:mjt]
                am2 = bias_t[:, mjt:2 * mjt]
                bq = bias_t[:, 2 * mjt:2 * mjt + 1]
                bk = bias_t[:, 2 * mjt + 1:2 * mjt + 2]
                cT = [pin.tile([128, mp], F16, tag=f"cT{i}", name=f"cTm{i}") for i in range(2)]
                nc.sync.dma_start(cT[0][:, 0:512], d_cT[0][:, 0:512])
                nc.gpsimd.dma_start(cT[1][:, 0:512], d_cT[1][:, 0:512])
                if c_chunk:
                    # two chunks per half: mid-loop S tiles unblock at the
                    # first chunk's completion instead of the whole rest
                    mid = 512 + (((mp - 512) // 2 + 127) // 128) * 128
                    nc.gpsimd.dma_start(cT[0][:, 512:mid], d_cT[0][:, 512:mid])
                    nc.scalar.dma_start(cT[1][:, 512:mid], d_cT[1][:, 512:mid])
                    nc.gpsimd.dma_start(cT[1][:, mid:mp], d_cT[1][:, mid:mp])
                    nc.scalar.dma_start(cT[0][:, mid:mp], d_cT[0][:, mid:mp])
                else:
                    nc.gpsimd.dma_start(cT[0][:, 512:mp], d_cT[0][:, 512:mp])
                    nc.scalar.dma_start(cT[1][:, 512:mp], d_cT[1][:, 512:mp])
                wo_t = pin.tile([128, DIM + D_HEAD], F16, tag="wo_t")
                nc.gpsimd.dma_start(wo_t[:], d_wo[:])
                wo = wo_t[:, 0:DIM]
                ones = wo_t[:, DIM:DIM + D_HEAD].bitcast(BF16)
            elif dma_split:
                # parallelize the head loads across all three DGE paths so
                # the first exp isn't gated on one queue draining ~1MB
                for ct in range(2):
                    nc.sync.dma_start(wq[ct][:], d_wq[ct])
                nc.sync.dma_start(xT[0][:, 0:512], d_xT[0][:, 0:512])
                nc.scalar.dma_start(xT[1][:, 0:512], d_xT[1][:, 0:512])
                nc.sync.dma_start(bq[:], d_bq[:])
                nc.sync.dma_start(xT[0][:, 512:1024], d_xT[0][:, 512:1024])
                nc.scalar.dma_start(xT[1][:, 512:1024], d_xT[1][:, 512:1024])
            else:
                for ct in range(2):
                    nc.sync.dma_start(wq[ct][:], d_wq[ct])
                    nc.sync.dma_start(xT[ct][:, 0:512], d_xT[ct][:, 0:512])
                nc.sync.dma_start(bq[:], d_bq[:])
                for ct in range(2):
                    nc.sync.dma_start(xT[ct][:, 512:1024], d_xT[ct][:, 512:1024])
            if not dma_merge:
              am = pin.tile([128, mjt], F32)
              nc.sync.dma_start(am[:], d_am[:])
              am2 = pin.tile([128, mjt], F32)
              nc.sync.dma_start(am2[:], d_am2[:])
              bk = pin.tile([128, 1], F32)
              nc.sync.dma_start(bk[:], d_bk[:])
              cT = [pin.tile([128, mp], F16, tag=f"cT{i}", name=f"cT{i}") for i in range(2)]
              if dma_split:
                nc.sync.dma_start(wk[0][:], d_wk[0])
                nc.scalar.dma_start(wk[1][:], d_wk[1])
                nc.sync.dma_start(cT[0][:, 0:512], d_cT[0][:, 0:512])
                nc.gpsimd.dma_start(cT[1][:, 0:512], d_cT[1][:, 0:512])
              else:
                for ct in range(2):
                    nc.sync.dma_start(wk[ct][:], d_wk[ct])
                    # first context chunk early: it gates S(jt0) and the ACT ramp
                    nc.sync.dma_start(cT[ct][:, 0:512], d_cT[ct][:, 0:512])
              ones = pin.tile([128, D_HEAD], BF16)
              nc.gpsimd.dma_start(ones[:], d_ones[:])
              for ct in range(2):
                nc.gpsimd.dma_start(wv[ct][:], d_wv[ct])
              if dma_split:
                # spread the bulk cT load across the other DGE paths so no
                # single queue serializes ~1MB of context
                qs = [nc.gpsimd, nc.scalar]
                qi = 0
                for c0 in range(512, mp, 512):
                    c1 = min(c0 + 512, mp)
                    for ct in range(2):
                        qs[qi % len(qs)].dma_start(cT[ct][:, c0:c1], d_cT[ct][:, c0:c1])
                        qi += 1
              else:
                for c0 in range(512, mp, 512):
                    # chunked so early j-tiles unblock before the whole load
                    c1 = min(c0 + 512, mp)
                    for ct in range(2):
                        nc.gpsimd.dma_start(cT[ct][:, c0:c1], d_cT[ct][:, c0:c1])
              wo = pin.tile([128, DIM], F16)
              nc.gpsimd.dma_start(wo[:], d_wo[:])

            # ---- persistent SBUF working tensors ----
            qT_hi = pwork.tile([128, N_Q], F16)   # [head*dim, i]  (pre-scaled by C1)
            kT_hi = pwork.tile([128, mp], F16)    # [head*dim, j]
            vnat = pwork.tile([128, mp], BF16)    # [j_local, jt*128 + head*dim]
            attnT = pwork.tile([128, N_Q], F16)
            linv = pwork.tile([128, N_Q], F32)
            outT = [pwork.tile([128, N_Q], F16, tag=f"outT{i}", name=f"outT{i}")
                    for i in range(2)]

            # warm the ACT exp table set during the DMA phase
            warm = pwork.tile([128, 1], F32, tag="warm")
            nc.vector.memset(warm[:], 0.0)
            warm2 = pwork.tile([128, 1], F32, tag="warm2")
            nc.scalar.activation(warm2[:], warm[:], AF.Exp)

            if pe_warm:
                # ~3.4us of dummy matmuls during the load phase flips the
                # HAM clock gate to 8/8 before the real PE work starts
                wps = ps_s.tile([128, 512 if ps512 else N_Q], F32, tag="s", name="wps")
                for i in range(pe_warm):
                    nc.tensor.matmul(wps[:, 0:128], wq[0][:], wq[1][:],
                                     start=True, stop=True)

            # ---- persistent PSUM accumulators (explicitly zeroed) ----
            pv_acc = ps_acc.tile([128, N_Q], F32, tag="pv")
            l_acc = ps_acc.tile([128, N_Q], F32, tag="l")
            nc.vector.memset(pv_acc[:], 0.0)
            nc.vector.memset(l_acc[:], 0.0)

            # ---- Q^T projection: qT = Wq^T @ x^T (+bq) ----
            for ih in range(2):
                ps = ps_s.tile([128, 512 if ps512 else N_Q], F32, tag="s", name="ps_q")
                sl = slice(ih * 512, ih * 512 + 512)
                for ct in range(2):
                    nc.tensor.matmul(
                        ps[:, 0:512], wq[ct][:], xT[ct][:, sl],
                        start=(ct == 0), stop=(ct == 1),
                    )
                nc.vector.tensor_scalar_add(qT_hi[:, sl], ps[:, 0:512], bq[:])

            # ---- main loop over j-tiles (software-pipelined: PV/L of the
            # previous tile are emitted AFTER the current tile's S matmuls,
            # so the next exp is never blocked behind them) ----
            def emit_pv(j0_p, e_prev, last):
                if strip_pvl:
                    return
                for ih in range(2):
                    sl = slice(ih * 512, ih * 512 + 512)
                    for h in range(HPC):
                        nc.tensor.matmul(
                            pv_acc[32 * h:32 * h + 32, sl],
                            vnat[:, j0_p + 32 * h:j0_p + 32 * h + 32],
                            e_prev[h][:, sl],
                            start=False, stop=(last and h == HPC - 1),
                            tile_position=(0, 32 * h),
                            skip_group_check=True,
                        )
                    for h in range(HPC):
                        nc.tensor.matmul(
                            l_acc[32 * h:32 * h + 32, sl],
                            ones[:],
                            e_prev[h][:, sl],
                            start=False, stop=(last and h == HPC - 1),
                            tile_position=(0, 32 * h),
                            skip_group_check=True,
                        )

            if k_batch:
                # whole K^T projection as a prologue (512-wide groups):
                # removes the per-tile K->kT-add->S cross-engine chain and
                # shrinks 17 small DVE bias-adds to 5 large ones
                for g0 in range(0, mp, 512):
                    g1 = min(g0 + 512, mp)
                    psk = ps_s.tile([128, 512 if ps512 else N_Q], F32,
                                    tag="s", name="psk")
                    for ct in range(2):
                        nc.tensor.matmul(
                            psk[:, 0:g1 - g0], wk[ct][:], cT[ct][:, g0:g1],
                            start=(ct == 0), stop=(ct == 1),
                        )
                    nc.vector.tensor_scalar_add(
                        kT_hi[:, g0:g1], psk[:, 0:g1 - g0], bk[:])

            def emit_kv(jt):
                # K^T and V projections for j-tile jt; the DVE copy/bias ops
                # are emitted here so they precede later exp ops in the DVE
                # FIFO and never gate the next tile's S matmuls
                j0 = jt * 128
                s_w = 512 if ps512 else N_Q
                if not k_batch:
                    ps = ps_s.tile([128, s_w], F32, tag="s", name="ps_k")
                    for ct in range(2):
                        nc.tensor.matmul(
                            ps[:, 0:128], wk[ct][:], cT[ct][:, j0:j0 + 128],
                            start=(ct == 0), stop=(ct == 1),
                        )
                    nc.vector.tensor_scalar_add(kT_hi[:, j0:j0 + 128], ps[:, 0:128], bk[:])

                psv = ps_s.tile([128, s_w], F32, tag="s", name="psv")
                for ct in range(2):
                    nc.tensor.matmul(
                        psv[:, 0:HD], cT[ct][:, j0:j0 + 128], wv[ct][:],
                        start=(ct == 0), stop=(ct == 1),
                    )
                nc.vector.tensor_copy(vnat[:, j0:j0 + 128], psv[:, 0:HD])

            if kv_ahead:
                emit_kv(0)
            pending = []  # [(j0, e_tiles)] awaiting their PV/L emission
            for jt in range(mjt):
                j0 = jt * 128
                if kv_ahead:
                    if jt + 1 < mjt:
                        emit_kv(jt + 1)
                else:
                    emit_kv(jt)

                # S^T + exp per head (hybrid ACT / DVE-Schraudolph)
                e_tiles = []
                for h in range(HPC):
                    hp = slice(32 * h, 32 * h + 32)
                    e_t = pe_pool.tile([128, N_Q], BF16, tag="e")
                    if ps512:
                        for ih in range(2):
                            sl = slice(ih * 512, ih * 512 + 512)
                            s_ps = ps_s.tile([128, 512], F32, tag="s")
                            nc.tensor.matmul(
                                s_ps[:], kT_hi[hp, j0:j0 + 128], qT_hi[hp, sl],
                                start=True, stop=True,
                                tile_position=(32 * h, 0),
                            )
                            if strip_exp:
                                nc.vector.memset(e_t[:, sl], 0.00390625)
                            elif (jt, h) in dve_set:
                                nc.vector.tensor_scalar(
                                    e_t[:, sl].bitcast(I16), s_ps[:],
                                    am2[:, jt:jt + 1], None,
                                    op0=mybir.AluOpType.add,
                                )
                            else:
                                nc.scalar.activation(
                                    e_t[:, sl], s_ps[:], AF.Exp,
                                    bias=am[:, jt:jt + 1], scale=1.0 / C1,
                                )
                        e_tiles.append(e_t)
                        continue
                    s_ps = ps_s.tile([128, N_Q], F32, tag="s")
                    for ih in range(2):
                        sl = slice(ih * 512, ih * 512 + 512)
                        nc.tensor.matmul(
                            s_ps[:, sl],
                            kT_hi[hp, j0:j0 + 128],
                            qT_hi[hp, sl],
                            start=True, stop=True,
                            tile_position=(32 * h, 0),
                        )
                    if strip_exp:
                        nc.vector.memset(e_t[:], 0.00390625)
                    elif (jt, h) in dve_set:
                        nc.vector.tensor_scalar(
                            e_t[:].bitcast(I16), s_ps[:],
                            am2[:, jt:jt + 1], None,
                            op0=mybir.AluOpType.add,
                        )
                    else:
                        nc.scalar.activation(
                            e_t[:], s_ps[:], AF.Exp,
                            bias=am[:, jt:jt + 1], scale=1.0 / C1,
                        )
                    e_tiles.append(e_t)

                # P @ V and row-sums for an EARLIER tile (pv_delay tiles
                # back, so its e inputs are final and the PE never stalls
                # on the exp engines behind queued S matmuls)
                pending.append((j0, e_tiles))
                if len(pending) > pv_delay:
                    pj, pe_t = pending.pop(0)
                    emit_pv(pj, pe_t, last=False)
                if jt == mjt - 1:
                    # flush the rest; the first hides under this tile's exps
                    while pending:
                        pj, pe_t = pending.pop(0)
                        emit_pv(pj, pe_t, last=(not pending))


            # ---- debug dumps ----
            if debug:
                dbg_l_s = pwork.tile([128, N_Q], F32, tag="dbg_l_s")
                nc.vector.tensor_copy(dbg_l_s[:], l_acc[:])
                nc.sync.dma_start(d_dbg["l"][:], dbg_l_s[:])
                dbg_pv_s = pwork.tile([128, N_Q], F32, tag="dbg_pv_s")
                nc.vector.tensor_copy(dbg_pv_s[:], pv_acc[:])
                nc.sync.dma_start(d_dbg["pv"][:], dbg_pv_s[:])
                dbg_q_s = pwork.tile([128, N_Q], F32, tag="dbg_q_s")
                nc.vector.tensor_copy(dbg_q_s[:], qT_hi[:])
                nc.sync.dma_start(d_dbg["q"][:], dbg_q_s[:])
                dbg_k_s = pwork.tile([128, mp], F32, tag="dbg_k_s")
                nc.vector.tensor_copy(dbg_k_s[:], kT_hi[:])
                nc.sync.dma_start(d_dbg["k"][:], dbg_k_s[:])

            # ---- normalize + output projection, per i-half so the PE can
            # start projecting half 0 while the DVE still normalizes half 1
            for ih in range(2):
                sl = slice(ih * 512, ih * 512 + 512)
                nc.vector.reciprocal(linv[:, sl], l_acc[:, sl])
                nc.vector.tensor_tensor(
                    attnT[:, sl], pv_acc[:, sl], linv[:, sl], mybir.AluOpType.mult)
                for dt in range(2):
                    ps = ps_s.tile([128, 512 if ps512 else N_Q], F32, tag="s", name="ps_o")
                    nc.tensor.matmul(
                        ps[:, 0:512], wo[:, dt * 128:dt * 128 + 128], attnT[:, sl],
                        start=True, stop=True,
                    )
                    if tail_act:
                        # ACT is idle after its last exp: let it do the
                        # psum->f16 copies so the DVE tail (recip+mult)
                        # runs in parallel with them
                        nc.scalar.activation(outT[dt][:, sl], ps[:, 0:512], AF.Copy)
                    else:
                        nc.vector.tensor_copy(outT[dt][:, sl], ps[:, 0:512])
                    nc.sync.dma_start(d_out[dt][:, sl], outT[dt][:, sl])
            if debug:
                nc.sync.dma_start(d_dbg["at"][:], attnT[:])

    nc.compile()
    return nc


def build_in_maps(inputs, keeps, mjt):
    x = np.ascontiguousarray(np.asarray(inputs["x"], dtype=np.float32))
    context = np.ascontiguousarray(np.asarray(inputs["context"], dtype=np.float32))
    frag_mask = np.asarray(inputs["frag_mask"], dtype=np.float32)
    W_qkv = np.ascontiguousarray(np.asarray(inputs["W_qkv"], dtype=np.float32))
    b_qkv = np.asarray(inputs["b_qkv"], dtype=np.float32)
    W_out = np.ascontiguousarray(np.asarray(inputs["W_out"], dtype=np.float32))
    mp = mjt * 128
    ones = np.ones((128, D_HEAD), dtype=ml_dtypes.bfloat16)
    in_maps = []
    for core in range(8):
        b, hh = core % B, core // B
        keep = keeps[b]
        cnt = len(keep)
        cT = np.zeros((DIM, mp), dtype=np.float32)
        cT[:, :cnt] = context[b][keep].T
        amask = np.full((mp,), NEG, dtype=np.float32)
        amask[:cnt] = frag_mask[b][keep]
        amask2 = amask * np.float32(C1) + np.float32(C2)
        hs = slice(hh * HD, (hh + 1) * HD)
        m = {
            "xT": np.ascontiguousarray(x[b].T.reshape(2, 128, N_Q)).astype(np.float16),
            "cT": np.ascontiguousarray(cT.reshape(2, 128, mp)).astype(np.float16),
            "wq": np.ascontiguousarray(W_qkv[:, hs].reshape(2, 128, HD) * np.float32(C1)).astype(np.float16),
            "wk": np.ascontiguousarray(W_qkv[:, 256:512][:, hs].reshape(2, 128, HD)).astype(np.float16),
            "wv": np.ascontiguousarray(W_qkv[:, 512:768][:, hs].reshape(2, 128, HD)).astype(np.float16),
            "wo": np.ascontiguousarray(W_out[hs, :]).astype(np.float16),
            "bq": np.ascontiguousarray(b_qkv[0:256][hs].reshape(128, 1) * np.float32(C1)),
            "bk": np.ascontiguousarray(b_qkv[256:512][hs].reshape(128, 1)),
            "amask": np.ascontiguousarray(amask.reshape(mjt, 128).T),
            "amask2": np.ascontiguousarray(amask2.reshape(mjt, 128).T),
            "ones": ones,
        }
        # merged-layout variants (dma_merge): superset keys are harmless
        m["wqkv"] = np.ascontiguousarray(
            np.concatenate([m["wq"], m["wk"], m["wv"]], axis=2))
        wo_m = np.zeros((128, DIM + D_HEAD), dtype=np.float16)
        wo_m[:, :DIM] = m["wo"]
        wo_m[:, DIM:] = ones.view(np.float16)[:, :D_HEAD]
        m["wo_ones"] = wo_m
        m["bias"] = np.ascontiguousarray(np.concatenate(
            [m["amask"], m["amask2"], m["bq"], m["bk"]], axis=1))
        in_maps.append(m)
    return in_maps


def kernel(x, context, mask, frag_mask, W_qkv, b_qkv, W_out, b_out):
    global last_results
    mask = np.asarray(mask).astype(bool)
    b_out = np.asarray(b_out, dtype=np.float32)

    keeps = [np.nonzero(mask[b])[0] for b in range(B)]
    mjt = max(1, max((len(k) + 127) // 128 for k in keeps))
    # DVE (Schraudolph) tiles must have no padded lanes in any batch
    dve_jt_max = min(len(k) // 128 for k in keeps)

    key = (mjt, 1, False)
    if key not in _cache:
        _cache[key] = _build(mjt, dve_jt_max=dve_jt_max, **BEST_CFG)
    nc = _cache[key]

    inputs = {"x": x, "context": context, "frag_mask": frag_mask,
              "W_qkv": W_qkv, "b_qkv": b_qkv, "W_out": W_out}
    in_maps = build_in_maps(inputs, keeps, mjt)

    res = run_bass_kernel_spmd(nc, in_maps, list(range(8)))
    last_results = res

    out = np.zeros((B, N_Q, DIM), dtype=np.float32)
    for core in range(8):
        b = core % B
        partial = res.results[core]["outT"].astype(np.float32).reshape(DIM, N_Q)
        out[b] += partial.T
    b_qkv = np.asarray(b_qkv, dtype=np.float32)
    out += (b_out + b_qkv[512:768] @ np.asarray(W_out, dtype=np.float32))[None, None, :]
    return out
